# revision 20
# baseline (speedup 1.0000x reference)
"""Causal self-attention (B=2, T=2048, C=1024, H=16) on 8 TRN2 NeuronCores.

Sharding: core c -> batch b = c // 4, head-group hg = c % 4 (4 heads each).
Each core computes q,k,v for its 4 heads, causal attention, and a partial
output projection (its 256 rows of w_proj). Host sums the 4 partials per
batch.

On-chip layout is fully "transposed" so no on-chip transposes are needed:
  - host passes xT = x[b].T  [C, T]
  - qT, kT computed as [head*64, T] (head-dim on partitions)
  - v computed as [T, head*65] where the 65th column per head is ones
  - scores computed transposed: sT[keys, queries] = kT_h^T-chunk @ qT_h
  - exp on ScalarE (no max subtraction: |logits/8| <= ~8, exp is safe in f32)
  - causal: fully-masked key-chunks skipped; diagonal chunks multiplied by a
    precomputed 0/1 band mask
  - PV matmul lhsT = v_aug[jchunk, head] [128, 65]: rows 0..63 accumulate
    y^T, row 64 (ones) accumulates the softmax denominator -- one matmul
  - normalize: reciprocal of the denominator row, broadcast across 64
    partitions with a K=1 matmul, multiply on VectorE
  - projection consumes y^T [c_in, t] directly as lhsT
"""

import numpy as np

import concourse.bacc as bacc
import concourse.mybir as mybir
import concourse.tile as tile
from concourse.bass_utils import run_bass_kernel_spmd

P = 128           # partitions
T = 2048          # sequence length
C = 1024          # model dim
NHC = 4           # heads per core
HD = 64           # head dim
JW = NHC * HD     # 256 qkv columns per core
VW = NHC * (HD + 1)  # 260: v + ones column per head
NCC = C // P      # 8 contraction chunks over C
NT = T // P       # 16 key/t chunks of 128
FI = 512          # query chunk (free dim of score matmuls)
NI = T // FI      # 4 query chunks

F32 = mybir.dt.float32
EXPF = mybir.ActivationFunctionType.Exp
COPYF = mybir.ActivationFunctionType.Copy
IDENT = mybir.ActivationFunctionType.Identity

# Matmul input dtype: float32r streams 1 row/cycle (vs 4 for float32) on the
# PE at free-dim >= 256. Storage is identical f32; only the PE pipeline
# differs.
MM_DT = mybir.dt.float32r


def build_nc(mm_dt=MM_DT, interleave=True, proj_pool=True,
             mm_bufs=2, s_bufs=None, p_bufs=4, o_bufs=2):
    nc = bacc.Bacc(
        "TRN2", target_bir_lowering=False, debug=False, enable_asserts=True
    )

    xt_d = nc.dram_tensor("xt", [C, T], F32, kind="ExternalInput")
    wq_d = nc.dram_tensor("wq", [C, JW], F32, kind="ExternalInput")
    wk_d = nc.dram_tensor("wk", [C, JW], F32, kind="ExternalInput")
    wv_d = nc.dram_tensor("wv", [C, JW], F32, kind="ExternalInput")
    bq_d = nc.dram_tensor("bq", [JW], F32, kind="ExternalInput")
    bk_d = nc.dram_tensor("bk", [JW], F32, kind="ExternalInput")
    bv_d = nc.dram_tensor("bv", [JW], F32, kind="ExternalInput")
    wp_d = nc.dram_tensor("wp", [JW, C], F32, kind="ExternalInput")
    ones_d = nc.dram_tensor("ones_c", [P], F32, kind="ExternalInput")
    mask_d = nc.dram_tensor("mask_c", [P, FI], F32, kind="ExternalInput")
    vones_d = nc.dram_tensor("vones_c", [P, NHC], F32, kind="ExternalInput")
    y_d = nc.dram_tensor("y", [T, C], F32, kind="ExternalOutput")

    # Tiles that feed matmuls are declared in the matmul dtype (float32r by
    # default -- same 4-byte storage as f32, but the BIR verifier requires
    # every producer of an FP32r matmul operand to emit float32r). DRAM stays
    # f32; DMA sources are bitcast so in/out dtypes agree.
    MMD = mm_dt

    def r(ap):  # matmul-operand view of an AP: ensure dtype == mm_dt
        if mm_dt == F32 or ap.dtype == mm_dt:
            return ap
        return ap.bitcast(mm_dt)

    with tile.TileContext(nc) as tc, \
            nc.allow_low_precision(reason="fp32r matmul operand tiles"):
        with (
            tc.tile_pool(name="big", bufs=1) as big,
            tc.tile_pool(name="pp", bufs=p_bufs) as p_pool,
            tc.tile_pool(name="op", bufs=o_bufs) as o_pool,
            tc.tile_pool(name="rows", bufs=2) as row_pool,
            tc.tile_pool(name="psmm", bufs=mm_bufs, space="PSUM") as ps_mm,
            tc.tile_pool(name="pss",
                         bufs=s_bufs if s_bufs is not None
                         else (3 if not proj_pool else 2),
                         space="PSUM") as ps_s,
            tc.tile_pool(name="pspv", bufs=2, space="PSUM") as ps_pv,
            tc.tile_pool(name="psbc", bufs=1, space="PSUM") as ps_bc,
            tc.tile_pool(name="pspj", bufs=1, space="PSUM") as ps_pj_real,
        ):
            ps_pj = ps_pj_real if proj_pool else ps_mm
            # --- constants (host-provided; memset can't write float32r) ---
            ones_row = big.tile([1, P], MMD, tag="ones", name="ones_row")
            nc.sync.dma_start(ones_row[:],
                              r(ones_d.ap().rearrange("(o f) -> o f", o=1)))
            # Diagonal score chunks are computed only on the valid column
            # suffix [rr:512), so every one masks with the same triangle:
            # maskb[j, u] = 1 if u >= j else 0 (sliced to [:, :512-rr]).
            maskb = big.tile([P, FI], MMD, tag="maskb", name="maskb")
            nc.sync.dma_start(maskb[:], r(mask_d.ap()[:, :]))
            vones = big.tile([P, NHC], MMD, tag="vones", name="vones")
            nc.sync.dma_start(vones[:], r(vones_d.ap()[:, :]))

            # --- biases ---
            bq_t, bk_t = {}, {}
            for kc in range(2):
                bqt = big.tile([P, 1], F32, tag=f"bq{kc}", name=f"bq{kc}")
                nc.sync.dma_start(
                    bqt[:],
                    bq_d.ap()[kc * P:(kc + 1) * P].rearrange("(p o) -> p o", o=1),
                )
                bq_t[kc] = bqt
                bkt = big.tile([P, 1], F32, tag=f"bk{kc}", name=f"bk{kc}")
                nc.sync.dma_start(
                    bkt[:],
                    bk_d.ap()[kc * P:(kc + 1) * P].rearrange("(p o) -> p o", o=1),
                )
                bk_t[kc] = bkt
            bv_row = big.tile([1, JW], MMD, tag="bv", name="bv_row")
            nc.sync.dma_start(bv_row[:], r(bv_d.ap().rearrange("(o f) -> o f", o=1)))

            # --- weights ---
            wq_t, wk_t, wv_t = {}, {}, {}
            for ci in range(NCC):
                for nm, d, store in (("wq", wq_d, wq_t), ("wk", wk_d, wk_t),
                                     ("wv", wv_d, wv_t)):
                    wt = big.tile([P, JW], MMD, tag=f"{nm}{ci}", name=f"{nm}{ci}")
                    nc.sync.dma_start(wt[:], r(d.ap()[ci * P:(ci + 1) * P, :]))
                    store[ci] = wt
            wp_t = {}
            for kc in range(2):
                wpt = big.tile([P, C], MMD, tag=f"wp{kc}", name=f"wp{kc}")
                nc.sync.dma_start(wpt[:], r(wp_d.ap()[kc * P:(kc + 1) * P, :]))
                wp_t[kc] = wpt

            yT = {}
            for kc in range(2):
                for ic in range(NI):
                    yt = big.tile([P, FI], MMD, tag=f"yT{kc}_{ic}",
                                  name=f"yT{kc}_{ic}")
                    yT[(kc, ic)] = yt

            xt_t, qT, kT, v_t = {}, {}, {}, {}

            def emit_xt_dma(ic):
                # xT for this t-window, per contraction chunk
                for ci in range(NCC):
                    xtt = big.tile([P, FI], MMD, tag=f"xt{ci}_{ic}",
                                   name=f"xt{ci}_{ic}")
                    nc.sync.dma_start(
                        xtt[:],
                        r(xt_d.ap()[ci * P:(ci + 1) * P,
                                    ic * FI:(ic + 1) * FI]),
                    )
                    xt_t[(ci, ic)] = xtt

            def emit_qk(ic, which):
                # qT or kT for this window
                for nm, w_t, b_t, store in [(("qT", wq_t, bq_t, qT),
                                             ("kT", wk_t, bk_t, kT))[which]]:
                    for kc in range(2):
                        ps = ps_mm.tile([P, FI], F32, tag="mm", name="ps_qk")
                        for ci in range(NCC):
                            nc.tensor.matmul(
                                ps[:],
                                r(w_t[ci][:, kc * P:(kc + 1) * P]),
                                r(xt_t[(ci, ic)][:]),
                                start=(ci == 0),
                                stop=(ci == NCC - 1),
                            )
                        st = big.tile([P, FI], MMD, tag=f"{nm}{kc}_{ic}",
                                      name=f"{nm}{kc}_{ic}")
                        nc.scalar.activation(st[:], ps[:], IDENT,
                                             bias=b_t[kc][:], scale=1.0)
                        store[(kc, ic)] = st

            def emit_v(ic, half):
                # v for 2 of the 4 t-chunks of this window
                for tc_i in range(4 * ic + 2 * half, 4 * ic + 2 * half + 2):
                    ps = ps_mm.tile([P, JW], F32, tag="mm", name="ps_v")
                    for ci in range(NCC):
                        nc.tensor.matmul(
                            ps[:],
                            r(xt_t[(ci, ic)][:, (tc_i % 4) * P:
                                             (tc_i % 4 + 1) * P]),
                            r(wv_t[ci][:]),
                            start=(ci == 0),
                            stop=False,
                        )
                    nc.tensor.matmul(ps[:], r(ones_row[:, :P]), r(bv_row[:]),
                                     start=False, stop=True)
                    vt = big.tile([P, VW], MMD, tag=f"v{tc_i}",
                                  name=f"v{tc_i}")
                    vt3 = vt.rearrange("p (h e) -> p h e", e=HD + 1)
                    nc.vector.tensor_copy(
                        vt3[:, :, 0:HD],
                        ps.rearrange("p (h e) -> p h e", e=HD),
                    )
                    nc.vector.tensor_copy(
                        vt3[:, :, HD:HD + 1],
                        vones.rearrange("p (h o) -> p h o", o=1),
                    )
                    v_t[tc_i] = vt

            def emit_qkv_piece(ic, piece):
                if piece == 0:
                    emit_qk(ic, 0)
                elif piece == 1:
                    emit_qk(ic, 1)
                else:
                    emit_v(ic, piece - 2)

            def emit_attention_head(ic, hh):
                # attention for query window ic (keys 0 .. 4*(ic+1) chunks)
                if True:
                    kc = hh // 2
                    po = (hh % 2) * HD
                    pv = ps_pv.tile([HD + 1, FI], F32, tag="pv", name="ps_pv")
                    njc = 4 * (ic + 1)
                    for jc in range(njc):
                        rr = jc * P - ic * FI  # key offset into query window
                        w = FI - rr if rr > 0 else FI  # valid column suffix
                        ss = ps_s.tile([P, FI], F32, tag="s", name="ps_s")
                        nc.tensor.matmul(
                            ss[:, :w],
                            r(kT[(kc, jc // 4)][po:po + HD,
                                                (jc % 4) * P:(jc % 4 + 1) * P]),
                            r(qT[(kc, ic)][po:po + HD, FI - w:]),
                            start=True,
                            stop=True,
                        )
                        pt = p_pool.tile([P, FI], MMD, tag="p", name="p_t")
                        nc.scalar.activation(pt[:, :w], ss[:, :w], EXPF,
                                             scale=0.125)
                        if rr >= 0:  # diagonal chunk: zero future keys
                            nc.vector.tensor_mul(
                                pt[:, :w], pt[:, :w], maskb[:, :w]
                            )
                        nc.tensor.matmul(
                            pv[:, FI - w:],
                            r(v_t[jc][:, hh * (HD + 1):(hh + 1) * (HD + 1)]),
                            r(pt[:, :w]),
                            start=(jc == 0),
                            stop=(jc == njc - 1),
                            skip_group_check=True,
                        )
                    rrow = row_pool.tile([1, FI], MMD, tag="rr", name="rrow")
                    nc.vector.reciprocal(rrow[:], pv[HD:HD + 1, :])
                    bc = ps_bc.tile([HD, FI], F32, tag="bc", name="ps_bc")
                    nc.tensor.matmul(bc[:], r(ones_row[:, :HD]), r(rrow[:]),
                                     start=True, stop=True)
                    ysl = yT[(kc, ic)][po:po + HD, :]
                    nc.scalar.activation(ysl, pv[0:HD, :], COPYF)
                    nc.vector.tensor_mul(ysl, ysl, bc[:])

            def emit_proj(ic):
                # projection for this query window (t chunks 4*ic .. 4*ic+3)
                for tc_i in range(4 * ic, 4 * (ic + 1)):
                    tof = (tc_i % 4) * P
                    for n2 in range(2):
                        ps = ps_pj.tile([P, FI], F32,
                                        tag="pj" if proj_pool else "mm",
                                        name="ps_o")
                        for kc in range(2):
                            nc.tensor.matmul(
                                ps[:],
                                r(yT[(kc, ic)][:, tof:tof + P]),
                                r(wp_t[kc][:, n2 * FI:(n2 + 1) * FI]),
                                start=(kc == 0),
                                stop=(kc == 1),
                            )
                        ot = o_pool.tile([P, FI], F32, tag="o", name="o_t")
                        nc.vector.tensor_copy(ot[:], ps[:])
                        nc.sync.dma_start(
                            y_d.ap()[tc_i * P:(tc_i + 1) * P,
                                     n2 * FI:(n2 + 1) * FI],
                            ot[:],
                        )

            def emit_qkv(ic):
                emit_xt_dma(ic)
                for piece in range(4):
                    emit_qkv_piece(ic, piece)

            if interleave == "fine":
                # QKV(ic+1) pieces slotted between attention heads of window
                # ic, giving the PE fill work while ScalarE runs exp.
                emit_qkv(0)
                for ic in range(NI):
                    if ic + 1 < NI:
                        emit_xt_dma(ic + 1)
                    for hh in range(NHC):
                        emit_attention_head(ic, hh)
                        if ic + 1 < NI:
                            emit_qkv_piece(ic + 1, hh)
                    emit_proj(ic)
            elif interleave:
                for ic in range(NI):
                    emit_qkv(ic)
                    for hh in range(NHC):
                        emit_attention_head(ic, hh)
                    emit_proj(ic)
            else:
                for ic in range(NI):
                    emit_qkv(ic)
                for ic in range(NI):
                    for hh in range(NHC):
                        emit_attention_head(ic, hh)
                    emit_proj(ic)



    nc.compile()
    return nc


_NC_CACHE = {}


def _get_nc(mm_dt=MM_DT):
    key = str(mm_dt)
    if key not in _NC_CACHE:
        _NC_CACHE[key] = build_nc(mm_dt)
    return _NC_CACHE[key]


def make_in_maps(x, w_attn, b_attn, w_proj, b_proj):
    x = np.asarray(x, dtype=np.float32)
    w_attn = np.asarray(w_attn, dtype=np.float32)
    b_attn = np.asarray(b_attn, dtype=np.float32)
    w_proj = np.asarray(w_proj, dtype=np.float32)
    b_proj = np.asarray(b_proj, dtype=np.float32)

    ones_c = np.ones((P,), dtype=np.float32)
    mask_c = (np.arange(FI)[None, :] >= np.arange(P)[:, None]).astype(
        np.float32)
    vones_c = np.ones((P, NHC), dtype=np.float32)

    in_maps = []
    for core in range(8):
        b = core // 4
        hg = core % 4
        sl = slice(JW * hg, JW * (hg + 1))
        in_maps.append({
            "ones_c": ones_c,
            "mask_c": mask_c,
            "vones_c": vones_c,
            "xt": np.ascontiguousarray(x[b].T),
            "wq": np.ascontiguousarray(w_attn[:, 0 * C:1 * C][:, sl]),
            "wk": np.ascontiguousarray(w_attn[:, 1 * C:2 * C][:, sl]),
            "wv": np.ascontiguousarray(w_attn[:, 2 * C:3 * C][:, sl]),
            "bq": np.ascontiguousarray(b_attn[0 * C:1 * C][sl]),
            "bk": np.ascontiguousarray(b_attn[1 * C:2 * C][sl]),
            "bv": np.ascontiguousarray(b_attn[2 * C:3 * C][sl]),
            "wp": np.ascontiguousarray(w_proj[sl, :]),
        })
    return in_maps


def _combine(parts, b_proj):
    y0 = parts[0] + parts[1] + parts[2] + parts[3]
    y1 = parts[4] + parts[5] + parts[6] + parts[7]
    y = np.stack([y0, y1], axis=0) + np.asarray(b_proj, np.float32)
    return y.astype(np.float32)


def run(x, w_attn, b_attn, w_proj, b_proj, trace=False, mm_dt=MM_DT):
    nc = _get_nc(mm_dt)
    in_maps = make_in_maps(x, w_attn, b_attn, w_proj, b_proj)
    res = run_bass_kernel_spmd(
        nc, in_maps, core_ids=list(range(8)), trace=trace
    )
    parts = [np.asarray(res.results[c]["y"]) for c in range(8)]
    return _combine(parts, b_proj), res


def kernel(x, w_attn, b_attn, w_proj, b_proj):
    y, _ = run(x, w_attn, b_attn, w_proj, b_proj, trace=False)
    return y


# ---------------------------------------------------------------------------
# Benchmark path: replicates bass2jax.run_bass_via_pjrt's multi-core dispatch
# but WITHOUT donation, so the jitted executable can be re-invoked on
# device-resident buffers to measure steady-state execution wall time.
# ---------------------------------------------------------------------------
def make_bench(x, w_attn, b_attn, w_proj, b_proj, mm_dt=MM_DT, n_cores=8):
    import jax
    import concourse.mybir as mb
    from concourse import bass2jax
    from jax.experimental.shard_map import shard_map
    from jax.sharding import Mesh, NamedSharding, PartitionSpec

    nc = _get_nc(mm_dt)
    in_maps = make_in_maps(x, w_attn, b_attn, w_proj, b_proj)
    bass2jax.install_neuronx_cc_hook()

    partition_name = (
        nc.partition_id_tensor.name if nc.partition_id_tensor else None
    )
    in_names, out_names, out_avals, zero_outs = [], [], [], []
    for alloc in nc.m.functions[0].allocations:
        if not isinstance(alloc, mb.MemoryLocationSet):
            continue
        name = alloc.memorylocations[0].name
        if alloc.kind == "ExternalInput":
            if name != partition_name:
                in_names.append(name)
        elif alloc.kind == "ExternalOutput":
            out_names.append(name)
            shape = tuple(alloc.tensor_shape)
            dtype = mb.dt.np(alloc.dtype)
            out_avals.append(jax.core.ShapedArray(shape, dtype))
            zero_outs.append(np.zeros(shape, dtype))
    n_params = len(in_names)
    all_names = in_names + out_names
    if partition_name is not None:
        all_names = all_names + [partition_name]

    def _body(*args):
        operands = list(args)
        if partition_name is not None:
            operands.append(bass2jax.partition_id_tensor())
        outs = bass2jax._bass_exec_p.bind(
            *operands,
            out_avals=tuple(out_avals),
            in_names=tuple(all_names),
            out_names=tuple(out_names),
            lowering_input_output_aliases=(),
            sim_require_finite=True,
            sim_require_nnan=True,
            nc=nc,
        )
        return tuple(outs)

    devices = jax.devices()[:n_cores]
    mesh = Mesh(np.asarray(devices), ("core",))
    spec = PartitionSpec("core")
    f = jax.jit(
        shard_map(
            _body, mesh=mesh,
            in_specs=(spec,) * (n_params + len(out_names)),
            out_specs=(spec,) * len(out_names),
            check_rep=False,
        ),
        keep_unused=True,
    )
    sharding = NamedSharding(mesh, spec)
    args = [
        jax.device_put(
            np.concatenate([np.asarray(m[nm]) for m in in_maps], axis=0),
            sharding,
        )
        for nm in in_names
    ] + [
        jax.device_put(
            np.zeros((n_cores * z.shape[0], *z.shape[1:]), z.dtype), sharding
        )
        for z in zero_outs
    ]
    return f, args, out_names


def bench(x, w_attn, b_attn, w_proj, b_proj, iters=30, mm_dt=MM_DT):
    import time

    import jax

    f, args, out_names = make_bench(x, w_attn, b_attn, w_proj, b_proj, mm_dt)
    out = f(*args)  # compile + warm
    jax.block_until_ready(out)
    times = []
    for _ in range(iters):
        t0 = time.perf_counter()
        out = f(*args)
        jax.block_until_ready(out)
        times.append(time.perf_counter() - t0)
    times.sort()
    y_all = np.asarray(out[out_names.index("y")]).reshape(8, T, C)
    y = _combine([y_all[c] for c in range(8)], b_proj)
    return y, times


# revision 26
# speedup vs baseline: 660.6316x; 660.6316x over previous
"""Causal self-attention (B=2, T=2048, C=1024, H=16) on 8 TRN2 NeuronCores.

Sharding: core c -> batch b = c // 4, head-group hg = c % 4 (4 heads each).
Each core computes q,k,v for its 4 heads, causal attention, and a partial
output projection (its 256 rows of w_proj). Host sums the 4 partials per
batch.

On-chip layout is fully "transposed" so no on-chip transposes are needed:
  - host passes xT = x[b].T  [C, T]
  - qT, kT computed as [head*64, T] (head-dim on partitions)
  - v computed as [T, head*65] where the 65th column per head is ones
  - scores computed transposed: sT[keys, queries] = kT_h^T-chunk @ qT_h
  - exp on ScalarE (no max subtraction: |logits/8| <= ~8, exp is safe in f32)
  - causal: fully-masked key-chunks skipped; diagonal chunks multiplied by a
    precomputed 0/1 band mask
  - PV matmul lhsT = v_aug[jchunk, head] [128, 65]: rows 0..63 accumulate
    y^T, row 64 (ones) accumulates the softmax denominator -- one matmul
  - normalize: reciprocal of the denominator row, broadcast across 64
    partitions with a K=1 matmul, multiply on VectorE
  - projection consumes y^T [c_in, t] directly as lhsT
"""

import numpy as np

import concourse.bacc as bacc
import concourse.mybir as mybir
import concourse.tile as tile
from concourse.bass_utils import run_bass_kernel_spmd

P = 128           # partitions
T = 2048          # sequence length
C = 1024          # model dim
NHC = 4           # heads per core
HD = 64           # head dim
JW = NHC * HD     # 256 qkv columns per core
VW = NHC * (HD + 1)  # 260: v + ones column per head
NCC = C // P      # 8 contraction chunks over C
NT = T // P       # 16 key/t chunks of 128
FI = 512          # query chunk (free dim of score matmuls)
NI = T // FI      # 4 query chunks

F32 = mybir.dt.float32
EXPF = mybir.ActivationFunctionType.Exp
COPYF = mybir.ActivationFunctionType.Copy
IDENT = mybir.ActivationFunctionType.Identity

# Matmul input dtype: float32r streams 1 row/cycle (vs 4 for float32) on the
# PE at free-dim >= 256. Storage is identical f32; only the PE pipeline
# differs.
MM_DT = mybir.dt.float32r


def build_nc(mm_dt=MM_DT, interleave="fine", proj_pool=False,
             mm_bufs=2, s_bufs=2, p_bufs=4, o_bufs=3, reps=1):
    nc = bacc.Bacc(
        "TRN2", target_bir_lowering=False, debug=False, enable_asserts=True
    )

    xt_d = nc.dram_tensor("xt", [C, T], F32, kind="ExternalInput")
    wq_d = nc.dram_tensor("wq", [C, JW], F32, kind="ExternalInput")
    wk_d = nc.dram_tensor("wk", [C, JW], F32, kind="ExternalInput")
    wv_d = nc.dram_tensor("wv", [C, JW], F32, kind="ExternalInput")
    bq_d = nc.dram_tensor("bq", [JW], F32, kind="ExternalInput")
    bk_d = nc.dram_tensor("bk", [JW], F32, kind="ExternalInput")
    bv_d = nc.dram_tensor("bv", [JW], F32, kind="ExternalInput")
    wp_d = nc.dram_tensor("wp", [JW, C], F32, kind="ExternalInput")
    ones_d = nc.dram_tensor("ones_c", [P], F32, kind="ExternalInput")
    mask_d = nc.dram_tensor("mask_c", [P, 2 * FI], F32, kind="ExternalInput")
    vones_d = nc.dram_tensor("vones_c", [P, NHC], F32, kind="ExternalInput")
    y_d = nc.dram_tensor("y", [T, C], F32, kind="ExternalOutput")

    # Tiles that feed matmuls are declared in the matmul dtype (float32r by
    # default -- same 4-byte storage as f32, but the BIR verifier requires
    # every producer of an FP32r matmul operand to emit float32r). DRAM stays
    # f32; DMA sources are bitcast so in/out dtypes agree.
    MMD = mm_dt

    def r(ap):  # matmul-operand view of an AP: ensure dtype == mm_dt
        if mm_dt == F32 or ap.dtype == mm_dt:
            return ap
        return ap.bitcast(mm_dt)

    with tile.TileContext(nc) as tc, \
            nc.allow_low_precision(reason="fp32r matmul operand tiles"):
        with (
            tc.tile_pool(name="big", bufs=1) as big,
            tc.tile_pool(name="pp", bufs=p_bufs) as p_pool,
            tc.tile_pool(name="op", bufs=o_bufs) as o_pool,
            tc.tile_pool(name="rows", bufs=2) as row_pool,
            tc.tile_pool(name="psmm", bufs=mm_bufs, space="PSUM") as ps_mm,
            # pair score tiles are [P, 2*FI] = 2 banks each
            tc.tile_pool(name="pss",
                         bufs=s_bufs if s_bufs is not None else 1,
                         space="PSUM") as ps_s,
            tc.tile_pool(name="pspv", bufs=2, space="PSUM") as ps_pv,
            tc.tile_pool(name="psbc", bufs=1, space="PSUM") as ps_bc,
            tc.tile_pool(name="pspj", bufs=1, space="PSUM") as ps_pj_real,
        ):
            ps_pj = ps_pj_real if proj_pool else ps_mm
            ps_bcp = ps_bc if proj_pool else ps_mm
            bc_tag = "bc" if proj_pool else "mm"
            # --- constant/weight loads, emitted lazily in compute-gated
            # order (first-window xT and wq first; wp only before proj) ---
            consts = {}

            def load_w(nm, d, store):
                for ci in range(NCC):
                    wt = big.tile([P, JW], MMD, tag=f"{nm}{ci}",
                                  name=f"{nm}{ci}")
                    nc.sync.dma_start(wt[:], r(d.ap()[ci * P:(ci + 1) * P, :]))
                    store[ci] = wt

            def load_biases():
                for kc in range(2):
                    bqt = big.tile([P, 1], F32, tag=f"bq{kc}", name=f"bq{kc}")
                    nc.sync.dma_start(
                        bqt[:],
                        bq_d.ap()[kc * P:(kc + 1) * P]
                        .rearrange("(p o) -> p o", o=1),
                    )
                    bq_t[kc] = bqt
                    bkt = big.tile([P, 1], F32, tag=f"bk{kc}", name=f"bk{kc}")
                    nc.sync.dma_start(
                        bkt[:],
                        bk_d.ap()[kc * P:(kc + 1) * P]
                        .rearrange("(p o) -> p o", o=1),
                    )
                    bk_t[kc] = bkt

            def load_consts():
                ones_row = big.tile([1, P], MMD, tag="ones", name="ones_row")
                nc.sync.dma_start(
                    ones_row[:],
                    r(ones_d.ap().rearrange("(o f) -> o f", o=1)))
                consts["ones_row"] = ones_row
                # maskb[j, u] = 1 if u >= j else 0, duplicated for head pairs
                maskb = big.tile([P, 2 * FI], MMD, tag="maskb", name="maskb")
                nc.sync.dma_start(maskb[:], r(mask_d.ap()[:, :]))
                consts["maskb2"] = maskb.rearrange("p (h f) -> p h f", h=2)
                vones = big.tile([P, NHC], MMD, tag="vones", name="vones")
                nc.sync.dma_start(vones[:], r(vones_d.ap()[:, :]))
                consts["vones"] = vones
                bv_row = big.tile([1, JW], MMD, tag="bv", name="bv_row")
                nc.sync.dma_start(
                    bv_row[:], r(bv_d.ap().rearrange("(o f) -> o f", o=1)))
                consts["bv_row"] = bv_row

            def load_wp():
                for kc in range(2):
                    wpt = big.tile([P, C], MMD, tag=f"wp{kc}", name=f"wp{kc}")
                    nc.sync.dma_start(wpt[:],
                                      r(wp_d.ap()[kc * P:(kc + 1) * P, :]))
                    wp_t[kc] = wpt

            bq_t, bk_t = {}, {}
            wq_t, wk_t, wv_t, wp_t = {}, {}, {}, {}

            yT = {}
            for kc in range(2):
                for ic in range(NI):
                    yt = big.tile([P, FI], MMD, tag=f"yT{kc}_{ic}",
                                  name=f"yT{kc}_{ic}")
                    yT[(kc, ic)] = yt

            xt_t, qT, kT, v_t = {}, {}, {}, {}

            def emit_xt_dma(ic):
                # xT for this t-window, per contraction chunk
                for ci in range(NCC):
                    xtt = big.tile([P, FI], MMD, tag=f"xt{ci}_{ic}",
                                   name=f"xt{ci}_{ic}")
                    nc.sync.dma_start(
                        xtt[:],
                        r(xt_d.ap()[ci * P:(ci + 1) * P,
                                    ic * FI:(ic + 1) * FI]),
                    )
                    xt_t[(ci, ic)] = xtt

            def emit_qk(ic, which):
                # qT or kT for this window
                for nm, w_t, b_t, store in [(("qT", wq_t, bq_t, qT),
                                             ("kT", wk_t, bk_t, kT))[which]]:
                    for kc in range(2):
                        ps = ps_mm.tile([P, FI], F32, tag="mm", name="ps_qk")
                        for ci in range(NCC):
                            nc.tensor.matmul(
                                ps[:],
                                r(w_t[ci][:, kc * P:(kc + 1) * P]),
                                r(xt_t[(ci, ic)][:]),
                                start=(ci == 0),
                                stop=(ci == NCC - 1),
                            )
                        st = big.tile([P, FI], MMD, tag=f"{nm}{kc}_{ic}",
                                      name=f"{nm}{kc}_{ic}")
                        nc.scalar.activation(st[:], ps[:], IDENT,
                                             bias=b_t[kc][:], scale=1.0)
                        store[(kc, ic)] = st

            def emit_v(ic, half):
                # v for 2 of the 4 t-chunks of this window
                for tc_i in range(4 * ic + 2 * half, 4 * ic + 2 * half + 2):
                    ps = ps_mm.tile([P, JW], F32, tag="mm", name="ps_v")
                    for ci in range(NCC):
                        nc.tensor.matmul(
                            ps[:],
                            r(xt_t[(ci, ic)][:, (tc_i % 4) * P:
                                             (tc_i % 4 + 1) * P]),
                            r(wv_t[ci][:]),
                            start=(ci == 0),
                            stop=False,
                        )
                    nc.tensor.matmul(ps[:], r(consts["ones_row"][:, :P]),
                                     r(consts["bv_row"][:]),
                                     start=False, stop=True)
                    vt = big.tile([P, VW], MMD, tag=f"v{tc_i}",
                                  name=f"v{tc_i}")
                    vt3 = vt.rearrange("p (h e) -> p h e", e=HD + 1)
                    nc.vector.tensor_copy(
                        vt3[:, :, 0:HD],
                        ps.rearrange("p (h e) -> p h e", e=HD),
                    )
                    nc.vector.tensor_copy(
                        vt3[:, :, HD:HD + 1],
                        consts["vones"].rearrange("p (h o) -> p h o", o=1),
                    )
                    v_t[tc_i] = vt

            def emit_qkv_piece(ic, piece):
                if piece == 0:
                    emit_qk(ic, 0)
                elif piece == 1:
                    emit_qk(ic, 1)
                else:
                    emit_v(ic, piece - 2)

            def emit_attention_pair(ic, hp):
                # attention for query window ic, heads (2*hp, 2*hp+1): both
                # live in partition rows of the kc=hp qT/kT tiles, so their
                # score chunks share one [P, 2*FI] psum tile and ONE exp and
                # mask op each ([P, 2, w] strided APs).
                kc = hp
                njc = 4 * (ic + 1)
                pv = {}
                for sub in range(2):
                    pv[sub] = ps_pv.tile([HD + 1, FI], F32, tag="pv",
                                         name="ps_pv")
                for jc in range(njc):
                    rr = jc * P - ic * FI  # key offset into query window
                    w = FI - rr if rr > 0 else FI  # valid column suffix
                    ss = ps_s.tile([P, 2 * FI], F32, tag="s", name="ps_s")
                    for sub in range(2):
                        nc.tensor.matmul(
                            ss[:, sub * FI:sub * FI + w],
                            r(kT[(kc, jc // 4)][sub * HD:(sub + 1) * HD,
                                                (jc % 4) * P:(jc % 4 + 1) * P]),
                            r(qT[(kc, ic)][sub * HD:(sub + 1) * HD, FI - w:]),
                            start=True,
                            stop=True,
                        )
                    ss3 = ss.rearrange("p (h f) -> p h f", h=2)
                    pt = p_pool.tile([P, 2 * FI], MMD, tag="p", name="p_t")
                    pt3 = pt.rearrange("p (h f) -> p h f", h=2)
                    nc.scalar.activation(pt3[:, :, :w], ss3[:, :, :w], EXPF,
                                         scale=0.125)
                    if rr >= 0:  # diagonal chunk: zero future keys
                        nc.vector.tensor_mul(
                            pt3[:, :, :w], pt3[:, :, :w],
                            consts["maskb2"][:, :, :w]
                        )
                    for sub in range(2):
                        hh = 2 * hp + sub
                        nc.tensor.matmul(
                            pv[sub][:, FI - w:],
                            r(v_t[jc][:, hh * (HD + 1):(hh + 1) * (HD + 1)]),
                            r(pt[:, sub * FI:sub * FI + w]),
                            start=(jc == 0),
                            stop=(jc == njc - 1),
                            skip_group_check=True,
                        )
                for sub in range(2):
                    po = sub * HD
                    rrow = row_pool.tile([1, FI], MMD, tag="rr", name="rrow")
                    nc.vector.reciprocal(rrow[:], pv[sub][HD:HD + 1, :])
                    bc = ps_bcp.tile([HD, FI], F32, tag=bc_tag,
                                     name="ps_bc")
                    nc.tensor.matmul(bc[:], r(consts["ones_row"][:, :HD]),
                                     r(rrow[:]),
                                     start=True, stop=True)
                    ysl = yT[(kc, ic)][po:po + HD, :]
                    nc.vector.tensor_copy(ysl, pv[sub][0:HD, :])
                    nc.vector.tensor_mul(ysl, ysl, bc[:])

            def emit_proj(ic):
                # projection for this query window (t chunks 4*ic .. 4*ic+3)
                for tc_i in range(4 * ic, 4 * (ic + 1)):
                    tof = (tc_i % 4) * P
                    for n2 in range(2):
                        ps = ps_pj.tile([P, FI], F32,
                                        tag="pj" if proj_pool else "mm",
                                        name="ps_o")
                        for kc in range(2):
                            nc.tensor.matmul(
                                ps[:],
                                r(yT[(kc, ic)][:, tof:tof + P]),
                                r(wp_t[kc][:, n2 * FI:(n2 + 1) * FI]),
                                start=(kc == 0),
                                stop=(kc == 1),
                            )
                        ot = o_pool.tile([P, FI], F32, tag="o", name="o_t")
                        nc.vector.tensor_copy(ot[:], ps[:])
                        nc.sync.dma_start(
                            y_d.ap()[tc_i * P:(tc_i + 1) * P,
                                     n2 * FI:(n2 + 1) * FI],
                            ot[:],
                        )

            def emit_qkv(ic):
                emit_xt_dma(ic)
                for piece in range(4):
                    emit_qkv_piece(ic, piece)

            def load_front():
                load_biases()
                load_w("wq", wq_d, wq_t)
                load_w("wk", wk_d, wk_t)
                load_w("wv", wv_d, wv_t)
                load_consts()

            for _rep in range(reps):
                if interleave == "fine":
                    # QKV(ic+1) pieces slotted between attention pairs of
                    # window ic: PE fill work while ScalarE runs exp.
                    if _rep == 0:
                        emit_xt_dma(0)
                        load_front()
                        for piece in range(4):
                            emit_qkv_piece(0, piece)
                        load_wp()
                    else:
                        emit_qkv(0)
                    for ic in range(NI):
                        if ic + 1 < NI:
                            emit_xt_dma(ic + 1)
                        for hp in range(2):
                            emit_attention_pair(ic, hp)
                            if ic + 1 < NI:
                                emit_qkv_piece(ic + 1, 2 * hp)
                                emit_qkv_piece(ic + 1, 2 * hp + 1)
                        emit_proj(ic)
                elif interleave:
                    if _rep == 0:
                        emit_xt_dma(0)
                        load_front()
                        load_wp()
                    for ic in range(NI):
                        if _rep == 0 and ic == 0:
                            for piece in range(4):
                                emit_qkv_piece(0, piece)
                        else:
                            emit_qkv(ic)
                        for hp in range(2):
                            emit_attention_pair(ic, hp)
                        emit_proj(ic)
                else:
                    if _rep == 0:
                        emit_xt_dma(0)
                        load_front()
                        load_wp()
                    for ic in range(NI):
                        if _rep == 0 and ic == 0:
                            for piece in range(4):
                                emit_qkv_piece(0, piece)
                        else:
                            emit_qkv(ic)
                    for ic in range(NI):
                        for hp in range(2):
                            emit_attention_pair(ic, hp)
                        emit_proj(ic)



    nc.compile()
    return nc


_NC_CACHE = {}


def _get_nc(mm_dt=MM_DT, **kw):
    key = (str(mm_dt), tuple(sorted(kw.items())))
    if key not in _NC_CACHE:
        _NC_CACHE[key] = build_nc(mm_dt, **kw)
    return _NC_CACHE[key]


def make_in_maps(x, w_attn, b_attn, w_proj, b_proj):
    x = np.asarray(x, dtype=np.float32)
    w_attn = np.asarray(w_attn, dtype=np.float32)
    b_attn = np.asarray(b_attn, dtype=np.float32)
    w_proj = np.asarray(w_proj, dtype=np.float32)
    b_proj = np.asarray(b_proj, dtype=np.float32)

    ones_c = np.ones((P,), dtype=np.float32)
    tri = (np.arange(FI)[None, :] >= np.arange(P)[:, None]).astype(np.float32)
    mask_c = np.concatenate([tri, tri], axis=1)  # duplicated for head pairs
    vones_c = np.ones((P, NHC), dtype=np.float32)

    in_maps = []
    for core in range(8):
        b = core // 4
        hg = core % 4
        sl = slice(JW * hg, JW * (hg + 1))
        in_maps.append({
            "ones_c": ones_c,
            "mask_c": mask_c,
            "vones_c": vones_c,
            "xt": np.ascontiguousarray(x[b].T),
            "wq": np.ascontiguousarray(w_attn[:, 0 * C:1 * C][:, sl]),
            "wk": np.ascontiguousarray(w_attn[:, 1 * C:2 * C][:, sl]),
            "wv": np.ascontiguousarray(w_attn[:, 2 * C:3 * C][:, sl]),
            "bq": np.ascontiguousarray(b_attn[0 * C:1 * C][sl]),
            "bk": np.ascontiguousarray(b_attn[1 * C:2 * C][sl]),
            "bv": np.ascontiguousarray(b_attn[2 * C:3 * C][sl]),
            "wp": np.ascontiguousarray(w_proj[sl, :]),
        })
    return in_maps


def _combine(parts, b_proj):
    y0 = parts[0] + parts[1] + parts[2] + parts[3]
    y1 = parts[4] + parts[5] + parts[6] + parts[7]
    y = np.stack([y0, y1], axis=0) + np.asarray(b_proj, np.float32)
    return y.astype(np.float32)


def run(x, w_attn, b_attn, w_proj, b_proj, trace=False, mm_dt=MM_DT):
    nc = _get_nc(mm_dt)
    in_maps = make_in_maps(x, w_attn, b_attn, w_proj, b_proj)
    res = run_bass_kernel_spmd(
        nc, in_maps, core_ids=list(range(8)), trace=trace
    )
    parts = [np.asarray(res.results[c]["y"]) for c in range(8)]
    return _combine(parts, b_proj), res


def kernel(x, w_attn, b_attn, w_proj, b_proj):
    y, _ = run(x, w_attn, b_attn, w_proj, b_proj, trace=False)
    return y


# ---------------------------------------------------------------------------
# Benchmark path: replicates bass2jax.run_bass_via_pjrt's multi-core dispatch
# but WITHOUT donation, so the jitted executable can be re-invoked on
# device-resident buffers to measure steady-state execution wall time.
# ---------------------------------------------------------------------------
def make_bench(x, w_attn, b_attn, w_proj, b_proj, mm_dt=MM_DT, n_cores=8,
               **build_kw):
    import jax
    import concourse.mybir as mb
    from concourse import bass2jax
    from jax.experimental.shard_map import shard_map
    from jax.sharding import Mesh, NamedSharding, PartitionSpec

    nc = _get_nc(mm_dt, **build_kw)
    in_maps = make_in_maps(x, w_attn, b_attn, w_proj, b_proj)
    bass2jax.install_neuronx_cc_hook()

    partition_name = (
        nc.partition_id_tensor.name if nc.partition_id_tensor else None
    )
    in_names, out_names, out_avals, zero_outs = [], [], [], []
    for alloc in nc.m.functions[0].allocations:
        if not isinstance(alloc, mb.MemoryLocationSet):
            continue
        name = alloc.memorylocations[0].name
        if alloc.kind == "ExternalInput":
            if name != partition_name:
                in_names.append(name)
        elif alloc.kind == "ExternalOutput":
            out_names.append(name)
            shape = tuple(alloc.tensor_shape)
            dtype = mb.dt.np(alloc.dtype)
            out_avals.append(jax.core.ShapedArray(shape, dtype))
            zero_outs.append(np.zeros(shape, dtype))
    n_params = len(in_names)
    all_names = in_names + out_names
    if partition_name is not None:
        all_names = all_names + [partition_name]

    def _body(*args):
        operands = list(args)
        if partition_name is not None:
            operands.append(bass2jax.partition_id_tensor())
        outs = bass2jax._bass_exec_p.bind(
            *operands,
            out_avals=tuple(out_avals),
            in_names=tuple(all_names),
            out_names=tuple(out_names),
            lowering_input_output_aliases=(),
            sim_require_finite=True,
            sim_require_nnan=True,
            nc=nc,
        )
        return tuple(outs)

    devices = jax.devices()[:n_cores]
    mesh = Mesh(np.asarray(devices), ("core",))
    spec = PartitionSpec("core")
    f = jax.jit(
        shard_map(
            _body, mesh=mesh,
            in_specs=(spec,) * (n_params + len(out_names)),
            out_specs=(spec,) * len(out_names),
            check_rep=False,
        ),
        keep_unused=True,
    )
    sharding = NamedSharding(mesh, spec)
    args = [
        jax.device_put(
            np.concatenate([np.asarray(m[nm]) for m in in_maps], axis=0),
            sharding,
        )
        for nm in in_names
    ] + [
        jax.device_put(
            np.zeros((n_cores * z.shape[0], *z.shape[1:]), z.dtype), sharding
        )
        for z in zero_outs
    ]
    return f, args, out_names


def bench(x, w_attn, b_attn, w_proj, b_proj, iters=30, mm_dt=MM_DT,
          **build_kw):
    import time

    import jax

    f, args, out_names = make_bench(x, w_attn, b_attn, w_proj, b_proj, mm_dt,
                                    **build_kw)
    out = f(*args)  # compile + warm
    jax.block_until_ready(out)
    times = []
    for _ in range(iters):
        t0 = time.perf_counter()
        out = f(*args)
        jax.block_until_ready(out)
        times.append(time.perf_counter() - t0)
    times.sort()
    y_all = np.asarray(out[out_names.index("y")]).reshape(8, T, C)
    y = _combine([y_all[c] for c in range(8)], b_proj)
    return y, times
